# revision 1
# baseline (speedup 1.0000x reference)
"""MoE top-2: single-NEFF pair-type-tiled design, on 8 TRN2 cores.

Every 128-token tile is *pure in expert-pair type*: all its tokens share the
same (e1, e2) routed pair.  Both experts' matmuls accumulate in separate PSUM
banks; gate scaling + combine happens at PSUM eviction with per-partition
scalars; output rows store directly in token order.  No y scratch round-trip,
no indirect DMA, no second NEFF.

SPMD trick: the static program is a fixed schedule of "seats" (slot pairs
into a 2S-slot weight buffer) x tile counts; per-core differences in which
pair types a core serves are absorbed by permuting/duplicating expert weights
into the slot arrangement host-side.  The seat pattern is the elementwise max
over cores, found by a small search (T=18 tiles/core for the staged data vs
the 17.875 lower bound).

Perf notes (measured): ~149.5us vs 215.3us two-NEFF baseline.  Matmul span
is stall-free at 216ns per [128x128]@[128x512] fp16 matmul (75 TF/s/core);
remaining overhead is ~7.5us fixed preamble, ~5us first-weight wait (eased
by ko-chunked seat-0 weights on two DMA rings + PE warm-up matmuls), and
~11.4us teardown (drains + semaphore clears).  Weight/x/out streams live on
separate DMA rings: rings are blocking FIFOs, so a store waiting on an
eviction must never queue ahead of a load another engine needs.  Per-
partition tables must be padded to 2KB DRAM lines (tiny-line DMAs cost
~60ns/line in descriptor overhead).

Self-contained: shapes hardcoded for B=16384, E=8, D=1024, O=1024, K=2.
"""

import os
import sys
import types
from itertools import combinations_with_replacement

sys.path.insert(0, "/opt/trn_rl_repo")

import ml_dtypes
import numpy as np

import concourse.bass as bass
import concourse.mybir as mybir
from concourse import bass_utils
from concourse.tile import TileContext

B, E, D, O = 16384, 8, 1024, 1024
N_CORES = 8
P = 128
KO = D // P  # contraction chunks
OT = 512  # one PSUM bank of fp32
NOT = O // OT
S_SEATS = 5  # seats per core; 2*S_SEATS weight slots resident in SBUF
GPAD = 512  # gate table padded to 2KB DRAM lines (tiny lines cost ~60ns each)

_DT_MAP = {
    "float16": (mybir.dt.float16, np.float16),
    "bfloat16": (mybir.dt.bfloat16, ml_dtypes.bfloat16),
    "float32r": (mybir.dt.float32r, np.float32),
    "float32": (mybir.dt.float32, np.float32),
}

MAX_WAITS = int(os.environ.get("MOE_MAX_WAITS", "1"))


def _patch_tile_drain():
    """Public-walrus workaround: walrus codegen rejects instructions carrying
    more than a couple of sync-wait commands.  Tile's add_semaphores can put
    several waits on one instruction (and the kernel-tail drain carries one
    per live processor).  Hoist excess waits onto single-wait nop carriers
    emitted just before the instruction on the same engine."""
    from concourse.tile import TileContext as TC
    from concourse.vector_clock import ScopedClock

    if getattr(TC, "_moe_drain_patched", False):
        return

    orig_add = TC._add_instruction

    def _add_instruction(self, inst):
        si = getattr(inst, "sync_info", None)
        waits = list(si.on_wait or []) if si is not None else []
        if len(waits) > MAX_WAITS:
            hoist = waits[: len(waits) - MAX_WAITS]
            keep = waits[len(waits) - MAX_WAITS :]
            for w in hoist:
                nop = mybir.InstNoOp(
                    name=self.nc.get_next_instruction_name(),
                    engine=inst.engine,
                    bass_nofuse=True,
                    sync_info=mybir.SyncInfo(on_wait=[w], on_update=[]),
                )
                orig_add(self, nop)
            inst.sync_info = mybir.SyncInfo(
                on_wait=keep, on_update=list(si.on_update or [])
            )
        orig_add(self, inst)

    def _drain_and_barrier(self, tick_clock, wait_clock):
        carrier = self.nc.sync.nop(nofuse=True)
        wait_clock.add_sem_waits(
            carrier.ins, ScopedClock({None: tick_clock.global_clock})
        )
        si = carrier.ins.sync_info
        waits = list(si.on_wait or []) if si is not None else []
        if len(waits) > 1:
            carrier.ins.sync_info = mybir.SyncInfo(
                on_wait=waits[:1], on_update=list(si.on_update or [])
            )
            for w in waits[1:]:
                extra = self.nc.sync.nop(nofuse=True)
                extra.ins.sync_info = mybir.SyncInfo(on_wait=[w], on_update=[])
        self.nc.sync.drain()
        self.nc.all_engine_barrier()
        assert self.sems is not None
        popped = self.nc._tile_sem_poison_stack.pop()
        assert popped is self._sem_poison
        self.nc.clear_and_free_semaphores(list(self.sems.allocated().values()))
        self.nc.all_engine_barrier()

    TC._add_instruction = _add_instruction
    TC._drain_and_barrier = _drain_and_barrier
    TC._moe_drain_patched = True


def _find_pattern(demands):
    """Smallest identical-across-cores seat pattern (list of tile counts,
    desc) such that the type tile-demands pack into 8 copies of it, one type
    per (core, seat) bin, types splittable across bins."""

    def greedy_pack(pattern):
        bins = sorted(
            [(p, s) for s, p in enumerate(pattern) for _ in range(N_CORES)],
            reverse=True,
        )
        dem = sorted(((d, t) for t, d in demands.items()), reverse=True)
        chunks = []  # (type, bin_seat, n_tiles)
        for cap, seat in bins:
            if not dem:
                break
            dem.sort(reverse=True)
            d, t = dem[0]
            take = min(cap, d)
            chunks.append((t, seat, take))
            if take == d:
                dem.pop(0)
            else:
                dem[0] = (d - take, t)
        return chunks if not dem else None

    total = sum(demands.values())
    maxd = max(demands.values())
    t_lo = max((total + N_CORES - 1) // N_CORES, 1)
    for T in range(t_lo, total + 1):
        for pat in combinations_with_replacement(range(1, max(maxd, T) + 1), S_SEATS):
            if sum(pat) != T:
                continue
            pattern = sorted(pat, reverse=True)
            chunks = greedy_pack(pattern)
            if chunks is not None:
                return pattern, chunks
    raise AssertionError("unreachable: pattern search failed")


def _plan(gates):
    """Global dispatch plan.

    Returns (pattern, plans) where plans[c] = list over seats of
    (e1, e2, token_ids) with len(token_ids) <= pattern[s]*P  (empty seat:
    e1=0, e2=1, no tokens)."""
    ge1, ge2 = [], []
    for i in range(B):
        nz = np.nonzero(gates[i] > 0)[0]
        assert len(nz) == 2
        ge1.append(nz[0])
        ge2.append(nz[1])
    ge1 = np.array(ge1)
    ge2 = np.array(ge2)
    type_id = ge1 * E + ge2
    types, counts = np.unique(type_id, return_counts=True)
    tok_by_type = {int(t): np.nonzero(type_id == t)[0] for t in types}
    demands = {int(t): int(np.ceil(c / P)) for t, c in zip(types, counts)}
    pattern, chunks = _find_pattern(demands)

    # Distribute chunks to cores: per seat, hand chunks out round-robin.
    used = {int(t): 0 for t in types}  # tokens consumed per type
    seat_fill = {s: 0 for s in range(S_SEATS)}  # next core per seat
    plans = [[None] * S_SEATS for _ in range(N_CORES)]
    for t, seat, take in chunks:
        c = seat_fill[seat]
        seat_fill[seat] += 1
        toks = tok_by_type[t][used[t] : used[t] + take * P]
        used[t] += len(toks)
        assert plans[c][seat] is None
        plans[c][seat] = (t // E, t % E, toks)
    for c in range(N_CORES):
        for s in range(S_SEATS):
            if plans[c][s] is None:
                plans[c][s] = (0, 1, np.array([], np.int64))
    for t in types:
        assert used[int(t)] == len(tok_by_type[int(t)])
    return pattern, plans


def _build_core_inputs(x, gates, W, b, plan, pattern, np_dt):
    """plan = per-seat (e1, e2, toks).  Returns in_map plus the row->token
    layout used to scatter device rows back into the full output."""
    T = sum(pattern)
    layout = np.full((T * P,), -1, np.int64)
    g_arr = np.zeros((T * P, 2), np.float32)
    wsl = np.zeros((2 * S_SEATS, D, O), np_dt)
    t0 = 0
    for s in range(S_SEATS):
        e1, e2, toks = plan[s]
        n = len(toks)
        layout[t0 : t0 + n] = toks
        g_arr[t0 : t0 + n, 0] = gates[toks, e1]
        g_arr[t0 : t0 + n, 1] = gates[toks, e2]
        wsl[2 * s] = W[e1].astype(np_dt)
        wsl[2 * s + 1] = W[e2].astype(np_dt)
        t0 += pattern[s] * P
    xz = np.vstack([x.astype(np_dt), np.zeros((1, D), np_dt)])
    # xg[t, ki, ko, p] = x[layout[t*P+p], ko*128+ki]  (row -1 -> zeros)
    xg = xz[layout].reshape(T, P, KO, P).transpose(0, 3, 2, 1).copy()
    # wb[slot, h, ki, ko, oc] = W[slot_expert][ko*128+ki, h*OT+oc] — output
    # halves split host-side so every device DMA reads contiguous DRAM.
    wb = (
        wsl.reshape(2 * S_SEATS, KO, P, NOT, OT)
        .transpose(0, 3, 2, 1, 4)
        .copy()
    )
    g_dev = np.zeros((P, GPAD), np.float32)
    g_dev[:, : T * 2] = (
        g_arr.reshape(T, P, 2).transpose(1, 0, 2).reshape(P, T * 2)
    )
    bsl = np.zeros((1, 2 * S_SEATS, O), np_dt)
    for s in range(S_SEATS):
        e1, e2, _ = plan[s]
        bsl[0, 2 * s] = b[e1].astype(np_dt)
        bsl[0, 2 * s + 1] = b[e2].astype(np_dt)
    return {"xg": xg, "w": wb, "g": g_dev, "bvec": bsl}, layout


def _build_program(pattern, dt, ydt, bias_flag):
    """Single NEFF: per tile (type-pure, 128 tokens), accumulate both experts
    in separate PSUM banks, combine with gate scalars on eviction, store rows
    in token order."""
    T = sum(pattern)
    NS = 2 * S_SEATS
    nc = bass.Bass(target_bir_lowering=False, trn_type="TRN2")
    xg_d = nc.dram_tensor("xg", [T, P, KO, P], dt, kind="ExternalInput")
    w_d = nc.dram_tensor("w", [NS, NOT, P, KO, OT], dt, kind="ExternalInput")
    g_d = nc.dram_tensor("g", [P, GPAD], mybir.dt.float32, kind="ExternalInput")
    b_d = nc.dram_tensor("bvec", [1, NS, O], dt, kind="ExternalInput")
    out_d = nc.dram_tensor("out", [T * P, O], ydt, kind="ExternalOutput")

    with TileContext(nc) as tc:
        with (
            tc.tile_pool(name="const", bufs=1) as cpool,
            tc.tile_pool(name="wp", bufs=1) as wpool,
            tc.tile_pool(name="xp", bufs=7) as xpool,
            tc.tile_pool(name="yt", bufs=8) as ypool,
            tc.tile_pool(name="tp", bufs=4) as tpool,
            tc.tile_pool(name="ps", bufs=8, space="PSUM") as pspool,
        ):
            g_sb = cpool.tile([P, GPAD], mybir.dt.float32)
            if bias_flag:
                b_sb = cpool.tile([1, NS, O], dt)
                nc.sync.dma_start(out=b_sb[:], in_=b_d[:, :, :])
                ones_sb = cpool.tile([1, P], dt)
                nc.vector.memset(ones_sb[:], 1.0)

            # Weight slots: persistent tiles on the scalar engine's DMA ring
            # (x/out keep the sync ring), emitted in compute order so early
            # seats arrive first.  (gpsimd's ring measured slow for bulk
            # weight streaming; vector can't issue DMAs.)
            # First x tile up front so it leads the sync ring.
            x_first = xpool.tile([P, KO, P], dt, tag="x")
            nc.sync.dma_start(out=x_first[:], in_=xg_d[0, :, :, :])

            # PE warm-up: the tensor engine ramps to full speed only after
            # ~3us of continuous execution (first real matmuls otherwise run
            # at ~630ns instead of ~216ns cadence).  Burn the weight-wait
            # window on dummy matmuls over a memset tile.
            warm_sb = cpool.tile([P, 5 * P], dt)
            nc.vector.memset(warm_sb[:], 0.25)
            warm_ps = pspool.tile([P, OT], mybir.dt.float32, tag="ps")
            for _ in range(16):
                nc.tensor.matmul(
                    out=warm_ps[:],
                    lhsT=warm_sb[:, :P],
                    rhs=warm_sb[:, P:],
                    start=True,
                    stop=True,
                )

            # Weight slots: persistent [P, KO, OT] tiles per (seat, slot,
            # half), streamed in compute order (all contiguous DMAs thanks to
            # the host-side half-split layout).  Seat 0's h0 tiles stream in
            # 512KB ko-prefix chunks so the first matmuls start as soon as
            # the first chunk lands (subtile deps), with the b-slot chunks on
            # the sync ring right after x0 so both slots arrive in parallel.
            w_tiles = {}
            for s in range(S_SEATS):
                for ab in range(2):
                    wt = wpool.tile([P, NOT, KO, OT], dt, tag=f"w{s}_{ab}")
                    w_tiles[(s, ab)] = wt
            for ko0 in (0, KO // 2):
                kos = slice(ko0, ko0 + KO // 2)
                nc.scalar.dma_start(
                    out=w_tiles[(0, 0)][:, 0, kos, :], in_=w_d[0, 0, :, kos, :]
                )
            nc.scalar.dma_start(out=g_sb[:], in_=g_d[:, :])
            for ko0 in (0, KO // 2):
                kos = slice(ko0, ko0 + KO // 2)
                nc.sync.dma_start(
                    out=w_tiles[(0, 1)][:, 0, kos, :], in_=w_d[1, 0, :, kos, :]
                )
            for ab in range(2):
                nc.scalar.dma_start(
                    out=w_tiles[(0, ab)][:, 1], in_=w_d[ab, 1, :, :, :]
                )
            for s in range(1, S_SEATS):
                for h in range(NOT):
                    for ab in range(2):
                        nc.scalar.dma_start(
                            out=w_tiles[(s, ab)][:, h],
                            in_=w_d[2 * s + ab, h, :, :, :],
                        )

            # Per seat: sweep h0 over all tiles, then h1.  Tile 0's h0 needs
            # only the first 2MB of weights, and each h1 sweep starts a full
            # h0-sweep after its weights were requested — the tensor engine
            # (which executes in program order) never parks on a weight DMA
            # after the first ~6us.
            t = 0
            for s in range(S_SEATS):
                ks = pattern[s]
                xs, ys = [], []
                for k in range(ks):
                    if t + k == 0:
                        x_sb = x_first
                    else:
                        x_sb = xpool.tile([P, KO, P], dt, tag="x")
                        nc.sync.dma_start(out=x_sb[:], in_=xg_d[t + k, :, :, :])
                    xs.append(x_sb)
                    ys.append(ypool.tile([P, O], ydt, tag="y", name="y_sb"))
                for h in range(NOT):
                    for k in range(ks):
                        tt = t + k
                        ps_a = pspool.tile([P, OT], mybir.dt.float32, tag="ps")
                        ps_b = pspool.tile([P, OT], mybir.dt.float32, tag="ps")
                        for ab, ps in ((0, ps_a), (1, ps_b)):
                            wt = w_tiles[(s, ab)]
                            for ko in range(KO):
                                nc.tensor.matmul(
                                    out=ps[:],
                                    lhsT=xs[k][:, ko, :],
                                    rhs=wt[:, h, ko, :],
                                    start=(ko == 0),
                                    stop=(ko == KO - 1 and not bias_flag),
                                )
                            if bias_flag:
                                nc.tensor.matmul(
                                    out=ps[:],
                                    lhsT=ones_sb[:1, :],
                                    rhs=b_sb[:1, 2 * s + ab, h * OT : (h + 1) * OT],
                                    start=False,
                                    stop=True,
                                )
                        tmp = tpool.tile([P, OT], mybir.dt.float32, tag="tmp")
                        nc.vector.tensor_scalar_mul(
                            out=tmp[:],
                            in0=ps_a[:],
                            scalar1=g_sb[:, 2 * tt : 2 * tt + 1],
                        )
                        nc.vector.scalar_tensor_tensor(
                            out=ys[k][:, h * OT : (h + 1) * OT],
                            in0=ps_b[:],
                            scalar=g_sb[:, 2 * tt + 1 : 2 * tt + 2],
                            in1=tmp[:],
                            op0=mybir.AluOpType.mult,
                            op1=mybir.AluOpType.add,
                        )
                        # Store each half as soon as it's evicted: keeps the
                        # end-of-kernel chain (evict -> store -> drain) short.
                        nc.sync.dma_start(
                            out=out_d[tt * P : (tt + 1) * P, h * OT : (h + 1) * OT],
                            in_=ys[k][:, h * OT : (h + 1) * OT],
                        )
                t += ks
    return nc


def kernel(x, gates, W, b):
    _patch_tile_drain()
    dt_name = os.environ.get("MOE_DT", "float16")
    ydt_name = os.environ.get("MOE_YDT", "float16")
    dt, np_dt = _DT_MAP[dt_name]
    ydt, y_np_dt = _DT_MAP[ydt_name]
    bias_flag = bool(np.any(b != 0))

    gates = np.asarray(gates)
    x = np.ascontiguousarray(x)
    W = np.asarray(W)
    b = np.asarray(b)

    pattern, plans = _plan(gates)
    in_maps, layouts = [], []
    for c in range(N_CORES):
        m, layout = _build_core_inputs(x, gates, W, b, plans[c], pattern, np_dt)
        in_maps.append(m)
        layouts.append(layout)

    nc = _build_program(pattern, dt, ydt, bias_flag)

    trace = os.environ.get("MOE_TRACE", "0") == "1"
    kwargs = {}
    if trace:
        _install_ntff_shim()
        kwargs = dict(trace=True, trace_cores=list(range(N_CORES)))

    res = bass_utils.run_bass_kernel_spmd(
        nc, in_maps, core_ids=list(range(N_CORES)), **kwargs
    )
    if trace and res.exec_time_ns is not None:
        print(f"HW exec time: {res.exec_time_ns} ns "
              f"(mean {res.mean_exec_time_ns:.0f}; pattern {pattern})")
    out = np.empty((B, O), np.float32)
    for c in range(N_CORES):
        layout = layouts[c]
        valid = layout >= 0
        out[layout[valid]] = res.results[c]["out"][valid].astype(np.float32)
    return out


def _install_ntff_shim():
    """Best-effort: register the missing antenv.axon_hooks NTFF profile hook
    so trace=True yields exec_time_ns.  Only used when MOE_TRACE=1."""
    try:
        import antenv
        from trn_agent_boot.trn_boot import _ntff_profile_via_ctypes

        if "antenv.axon_hooks" in sys.modules:
            return
        hooks = types.ModuleType("antenv.axon_hooks")
        hook = _ntff_profile_via_ctypes("/opt/axon/libaxon_pjrt.so")
        hooks.get_axon_ntff_profile_hook = lambda: hook
        hooks.set_axon_ntff_profile_hook = lambda h: None
        sys.modules["antenv.axon_hooks"] = hooks
        antenv.axon_hooks = hooks
        bass_utils.upload_artifacts = lambda tmpdir: tmpdir
    except Exception as e:  # pragma: no cover
        print(f"ntff shim unavailable: {e}", file=sys.stderr)



# revision 3
# speedup vs baseline: 1.0305x; 1.0305x over previous
"""MoE top-2: transposed expert-parallel single-NEFF design, on 8 TRN2 cores.

Orientation: weights are the *stationary* matmul operand (lhsT = 128x128
W blocks), tokens stream as the *moving* operand (rhs = xT columns).  Each
core computes out.T[o, t] = W_e.T @ xT_e for the token-expert pairs it
owns, so cost is exact streamed columns (64 passes per column: 8 o-chunks
x 8 ko) instead of 128-token-padded tiles, and the gate folds into x on
the host ((g*x) @ W = g*(x @ W)) - no per-token scaling on device.

SPMD balance trick: the static program has 2 weight "slots" per core with
uniform chunk widths (slot A: 8x512 cols, slot B: 1 narrow chunk).  Light
experts sit whole in one core's A-bin; heavy experts put their first cA
tokens in an A-bin and spread the overflow over other cores' B-bins.
Which expert sits in which (core, slot) is a host-side input permutation;
the instruction stream is identical on all cores.  Capacities (cA, cB)
are solved at build time from the actual gate nonzero counts.

Chunk 0 runs ko-outer across all 8 PSUM banks so the first matmuls need
only the first 256KB weight row + first 128KB x slice (no multi-us
first-weight stall); remaining chunks run oc-outer with ko-inner
accumulation.  Slot-B evictions pack into one SBUF tile so the store is a
single contiguous DMA (tiny-line stores are descriptor-bound).  Rings:
x loads on sync, W on scalar, output stores on gpsimd; evictions
alternate vector/scalar.

Host combine: each (core, slot)-bin holds unique tokens of one expert, so
the full output is a plain fancy-index accumulate per bin; bias (if any)
is the rank-1 host add gates @ b.

Self-contained: shapes hardcoded for B=16384, E=8, D=1024, O=1024.
"""

import math
import os
import sys
import types

sys.path.insert(0, "/opt/trn_rl_repo")

import ml_dtypes
import numpy as np

import concourse.bass as bass
import concourse.mybir as mybir
from concourse import bass_utils
from concourse.tile import TileContext

B, E, D, O = 16384, 8, 1024, 1024
N_CORES = 8
P = 128
KO = D // P  # contraction blocks
NOC = O // P  # output partition chunks
OT = 512  # tokens per chunk == one fp32 PSUM bank

_DT_MAP = {
    "float16": (mybir.dt.float16, np.float16),
    "bfloat16": (mybir.dt.bfloat16, ml_dtypes.bfloat16),
    "float32": (mybir.dt.float32, np.float32),
}

MAX_WAITS = int(os.environ.get("MOE_MAX_WAITS", "1"))


def _patch_tile_drain():
    """Public-walrus workaround: walrus codegen rejects instructions carrying
    more than a couple of sync-wait commands.  Tile's add_semaphores can put
    several waits on one instruction (and the kernel-tail drain carries one
    per live processor).  Hoist excess waits onto single-wait nop carriers
    emitted just before the instruction on the same engine."""
    from concourse.tile import TileContext as TC
    from concourse.vector_clock import ScopedClock

    if getattr(TC, "_moe_drain_patched", False):
        return

    orig_add = TC._add_instruction

    def _add_instruction(self, inst):
        si = getattr(inst, "sync_info", None)
        waits = list(si.on_wait or []) if si is not None else []
        if len(waits) > MAX_WAITS:
            hoist = waits[: len(waits) - MAX_WAITS]
            keep = waits[len(waits) - MAX_WAITS :]
            for w in hoist:
                nop = mybir.InstNoOp(
                    name=self.nc.get_next_instruction_name(),
                    engine=inst.engine,
                    bass_nofuse=True,
                    sync_info=mybir.SyncInfo(on_wait=[w], on_update=[]),
                )
                orig_add(self, nop)
            inst.sync_info = mybir.SyncInfo(
                on_wait=keep, on_update=list(si.on_update or [])
            )
        orig_add(self, inst)

    def _drain_and_barrier(self, tick_clock, wait_clock):
        carrier = self.nc.sync.nop(nofuse=True)
        wait_clock.add_sem_waits(
            carrier.ins, ScopedClock({None: tick_clock.global_clock})
        )
        si = carrier.ins.sync_info
        waits = list(si.on_wait or []) if si is not None else []
        if len(waits) > 1:
            carrier.ins.sync_info = mybir.SyncInfo(
                on_wait=waits[:1], on_update=list(si.on_update or [])
            )
            for w in waits[1:]:
                extra = self.nc.sync.nop(nofuse=True)
                extra.ins.sync_info = mybir.SyncInfo(on_wait=[w], on_update=[])
        self.nc.sync.drain()
        self.nc.all_engine_barrier()
        assert self.sems is not None
        popped = self.nc._tile_sem_poison_stack.pop()
        assert popped is self._sem_poison
        self.nc.clear_and_free_semaphores(list(self.sems.allocated().values()))
        self.nc.all_engine_barrier()

    TC._add_instruction = _add_instruction
    TC._drain_and_barrier = _drain_and_barrier
    TC._moe_drain_patched = True


def _plan(ne):
    """Choose slot capacities and per-core bin assignment.

    Returns (cA, cB, plans): cA is a multiple of 512 (slot A capacity), cB
    the slot-B chunk width (0 = no B slot), plans[c] = dict with keys
    eA, tA0 (token offset into expert eA's list), nA, eB, tB0, nB."""
    total = sum(ne)
    cA = 512 * max(1, math.ceil(total / (N_CORES * 512)))
    while True:
        over = [max(0, n - cA) for n in ne]
        if not any(over):
            cB = 0
            break
        cB = None
        for cand in range(16, OT + 1, 16):
            if sum(math.ceil(o / cand) for o in over if o) <= N_CORES:
                cB = cand
                break
        if cB is not None:
            break
        cA += 512
    plans = [
        {"eA": e, "tA0": 0, "nA": min(ne[e], cA), "eB": None, "tB0": 0, "nB": 0}
        for e in range(N_CORES)
    ]
    core = 0
    for e in range(len(ne)):
        off = cA
        while off < ne[e]:
            take = min(cB, ne[e] - off)
            while plans[core]["eB"] is not None:
                core += 1
            plans[core].update(eB=e, tB0=off, nB=take)
            off += take
    return cA, cB, plans


def _build_program(nA_chunks, cB, dt, ydt):
    """Single NEFF shared by all cores.  Chunk schedule: nA_chunks x 512-col
    slot-A chunks + (if cB) one cB-col slot-B chunk."""
    has_b = cB > 0
    C = nA_chunks + (1 if has_b else 0)
    CW = C * OT
    nc = bass.Bass(target_bir_lowering=False, trn_type="TRN2")
    xg_d = nc.dram_tensor("xg", [C, KO, P, OT], dt, kind="ExternalInput")
    wa_d = nc.dram_tensor("wa", [KO, P, O], dt, kind="ExternalInput")
    if has_b:
        wb_d = nc.dram_tensor("wb", [KO, P, O], dt, kind="ExternalInput")
    ya_d = nc.dram_tensor("ya", [nA_chunks, NOC, P, OT], ydt, kind="ExternalOutput")
    if has_b:
        yb_d = nc.dram_tensor("yb", [P, NOC * cB], ydt, kind="ExternalOutput")

    with TileContext(nc) as tc:
        with (
            tc.tile_pool(name="wp", bufs=1) as wpool,
            tc.tile_pool(name="xp", bufs=1) as xpool,
            tc.tile_pool(name="yt", bufs=4) as ypool,
            tc.tile_pool(name="ybp", bufs=1) as ybpool,
            tc.tile_pool(name="ps", bufs=8, space="PSUM") as pspool,
        ):
            wa_t = wpool.tile([P, KO, O], dt, tag="wa")
            x_t = xpool.tile([P, KO, CW], dt, tag="x")
            if has_b:
                wb_t = wpool.tile([P, KO, O], dt, tag="wb")
                yb_t = ybpool.tile([P, NOC * cB], ydt, tag="yb")

            # Slot-A weights: one 256KB row per ko on the scalar ring, in
            # chunk-0's ko-outer consumption order.
            for ko in range(KO):
                nc.scalar.dma_start(out=wa_t[:, ko, :], in_=wa_d[ko])
            # All x chunks stream on the sync ring (whole x fits in SBUF).
            for c in range(C):
                for ko in range(KO):
                    nc.sync.dma_start(
                        out=x_t[:, ko, c * OT : (c + 1) * OT], in_=xg_d[c, ko]
                    )
            # Slot-B weights queue behind slot A; needed only near the end.
            if has_b:
                for ko in range(KO):
                    nc.scalar.dma_start(out=wb_t[:, ko, :], in_=wb_d[ko])

            def evict(i, out_ap, in_ap):
                if i % 2 == 0:
                    nc.vector.tensor_scalar_mul(out=out_ap, in0=in_ap, scalar1=1.0)
                else:
                    nc.scalar.copy(out=out_ap, in_=in_ap)

            # Chunk 0: ko-outer across all 8 PSUM banks, so the matmul for
            # (ko, oc) needs only W row ko / x slice ko - both land early.
            ps0 = [
                pspool.tile([P, OT], mybir.dt.float32, tag="ps", name=f"ps0_{i}")
                for i in range(NOC)
            ]
            for ko in range(KO):
                for oc in range(NOC):
                    nc.tensor.matmul(
                        out=ps0[oc][:],
                        lhsT=wa_t[:, ko, oc * P : (oc + 1) * P],
                        rhs=x_t[:, ko, 0:OT],
                        start=(ko == 0),
                        stop=(ko == KO - 1),
                    )
            for oc in range(NOC):
                y = ypool.tile([P, OT], ydt, tag="y")
                evict(oc, y[:], ps0[oc][:])
                nc.gpsimd.dma_start(out=ya_d[0, oc], in_=y[:])

            # Remaining slot-A chunks: oc-outer, ko-inner accumulation.
            for c in range(1, nA_chunks):
                for oc in range(NOC):
                    ps = pspool.tile([P, OT], mybir.dt.float32, tag="ps")
                    for ko in range(KO):
                        nc.tensor.matmul(
                            out=ps[:],
                            lhsT=wa_t[:, ko, oc * P : (oc + 1) * P],
                            rhs=x_t[:, ko, c * OT : (c + 1) * OT],
                            start=(ko == 0),
                            stop=(ko == KO - 1),
                        )
                    y = ypool.tile([P, OT], ydt, tag="y")
                    evict(oc, y[:], ps[:])
                    nc.gpsimd.dma_start(out=ya_d[c, oc], in_=y[:])

            # Slot-B chunk: narrow; evictions pack into one tile so the
            # store is a single contiguous DMA.
            if has_b:
                base = nA_chunks * OT
                for oc in range(NOC):
                    ps = pspool.tile([P, OT], mybir.dt.float32, tag="ps")
                    for ko in range(KO):
                        nc.tensor.matmul(
                            out=ps[:, :cB],
                            lhsT=wb_t[:, ko, oc * P : (oc + 1) * P],
                            rhs=x_t[:, ko, base : base + cB],
                            start=(ko == 0),
                            stop=(ko == KO - 1),
                        )
                    evict(oc, yb_t[:, oc * cB : (oc + 1) * cB], ps[:, :cB])
                nc.gpsimd.dma_start(out=yb_d[:], in_=yb_t[:])
    return nc


def kernel(x, gates, W, b):
    _patch_tile_drain()
    dt_name = os.environ.get("MOE_DT", "float16")
    ydt_name = os.environ.get("MOE_YDT", "float16")
    dt, np_dt = _DT_MAP[dt_name]
    ydt, _ = _DT_MAP[ydt_name]

    x = np.ascontiguousarray(np.asarray(x, np.float32))
    gates = np.asarray(gates, np.float32)
    W = np.asarray(W, np.float32)
    b = np.asarray(b, np.float32)

    toks = [np.nonzero(gates[:, e] > 0)[0] for e in range(E)]
    ne = [len(t) for t in toks]
    cA, cB, plans = _plan(ne)
    nA_chunks = cA // OT
    has_b = cB > 0
    C = nA_chunks + (1 if has_b else 0)

    def build_slot(e, t0, n, cap):
        """[KO, 128, cap] transposed gate-scaled x for one bin, fp16."""
        ids = toks[e][t0 : t0 + n]
        buf = np.zeros((cap, D), np.float32)
        buf[:n] = x[ids] * gates[ids, e][:, None]
        return np.ascontiguousarray(buf.T).astype(np_dt).reshape(KO, P, cap)

    in_maps = []
    for c in range(N_CORES):
        pl = plans[c]
        xg = np.zeros((C, KO, P, OT), np_dt)
        xa = build_slot(pl["eA"], pl["tA0"], pl["nA"], cA)
        xg[:nA_chunks] = xa.reshape(KO, P, nA_chunks, OT).transpose(2, 0, 1, 3)
        m = {"xg": xg, "wa": W[pl["eA"]].astype(np_dt).reshape(KO, P, O)}
        if has_b:
            eB = pl["eB"] if pl["eB"] is not None else pl["eA"]
            xg[nA_chunks, :, :, :cB] = build_slot(eB, pl["tB0"], pl["nB"], cB)
            m["wb"] = W[eB].astype(np_dt).reshape(KO, P, O)
        in_maps.append(m)

    nc = _build_program(nA_chunks, cB, dt, ydt)

    trace = os.environ.get("MOE_TRACE", "0") == "1"
    kwargs = {}
    if trace:
        _install_ntff_shim()
        kwargs = dict(trace=True, trace_cores=list(range(N_CORES)))

    res = bass_utils.run_bass_kernel_spmd(
        nc, in_maps, core_ids=list(range(N_CORES)), **kwargs
    )
    if trace and res.exec_time_ns is not None:
        print(f"HW exec time: {res.exec_time_ns} ns "
              f"(mean {res.mean_exec_time_ns:.0f}; cA {cA} cB {cB})")

    out = np.zeros((B, O), np.float32)
    for c in range(N_CORES):
        pl = plans[c]
        ya = res.results[c]["ya"]  # [nA_chunks, NOC, P, OT] (c, oc, p, w)
        arr = ya.transpose(0, 3, 1, 2).reshape(cA, O).astype(np.float32)
        ids = toks[pl["eA"]][pl["tA0"] : pl["tA0"] + pl["nA"]]
        out[ids] += arr[: pl["nA"]]
        if has_b and pl["eB"] is not None:
            yb = res.results[c]["yb"]  # [P, NOC*cB] (p, oc*cB + w)
            arrb = yb.reshape(P, NOC, cB).transpose(2, 1, 0).reshape(cB, O)
            ids = toks[pl["eB"]][pl["tB0"] : pl["tB0"] + pl["nB"]]
            out[ids] += arrb[: pl["nB"]].astype(np.float32)
    if np.any(b != 0):
        out += gates @ b
    return out


def _install_ntff_shim():
    """Best-effort: register the missing antenv.axon_hooks NTFF profile hook
    so trace=True yields exec_time_ns.  Only used when MOE_TRACE=1."""
    try:
        import antenv
        from trn_agent_boot.trn_boot import _ntff_profile_via_ctypes

        if "antenv.axon_hooks" in sys.modules:
            return
        hooks = types.ModuleType("antenv.axon_hooks")
        hook = _ntff_profile_via_ctypes("/opt/axon/libaxon_pjrt.so")
        hooks.get_axon_ntff_profile_hook = lambda: hook
        hooks.set_axon_ntff_profile_hook = lambda h: None
        sys.modules["antenv.axon_hooks"] = hooks
        antenv.axon_hooks = hooks
        bass_utils.upload_artifacts = lambda tmpdir: tmpdir
    except Exception as e:  # pragma: no cover
        print(f"ntff shim unavailable: {e}", file=sys.stderr)
